# revision 26
# baseline (speedup 1.0000x reference)
"""HaarMSELoss kernel for Trainium2 (8 NeuronCores, data-parallel).

Math: the 2x2 Haar transform used by the reference is (up to the 0.5
scaling) an orthogonal Hadamard transform, so for each 2x2 block
LL^2+LH^2+HL^2+HH^2 == a^2+b^2+c^2+d^2 of the block entries of
(input - target).  Hence

  loss = sum_bands mean((haar(x)-haar(y))^2)
       = sum((x-y)^2) / (B*C*(H/2)*(W/2))

a pure squared-difference reduction.  Each core reduces 1/8 of the
elements; the host sums the per-(partition,tile) partials (f64) and
divides.

Raw bass pipeline (per tile, slot s = t % NBUF):
  ACT : HWDGE dma loads of the x and y tile halves (qActDynamicHW ring);
        activation(Square, accum_out) -> stats[:, t] = sum(d^2).
        Slot-recycle safety is ACT program order: the refill dispatch for
        tile t+NBUF sits right after square(t), so no cross-engine gating
        waits are needed on the load path at all.
  DVE : d = x - y in place (x half of the slot)
  SP  : the two stats stores (bulk store overlaps the taper tail) +
        final completion wait.  Stores stay on SP: dispatching them from
        ACT immediately after the last square raced the accum write's
        drain to SBUF (observed rel err 0.145 once); the SP sem hop adds
        the latency that makes the read safe.

The first 7 tiles are 4096 floats per partition per operand and cycle
through NBUF=4 big slots; the last 4 tiles taper (2048, 1024, 768, 256)
and each own a dedicated buffer, so every load can be enqueued without
waiting on compute and the DMA queues stream back-to-back to the end.
The post-last-load sub+square+store tail is ~3 us instead of ~10 us.

DMA completion safety: the 16 SDMA engines each increment a DMA
semaphore by 1 at their own last descriptor, and engines drain their
rings independently, so a single cumulative semaphore across all tiles
can hit 16*(t+1) from fast engines' tile-(t+1) packets before a slow
engine has delivered tile t (observed as run-to-run result wobble in
the previous version).  Each buffer slot therefore gets its OWN
semaphore: generation g of a slot is only enqueued after the last
reader (ACT) finished generation g-1, so the 32*(g+1) threshold
(x dma +16, y dma +16) can only be reached once both of generation g's
transfers fully landed.
"""

import numpy as np

_B, _C, _H, _W = 4, 32, 512, 512
_TOTAL = _B * _C * _H * _W          # 33_554_432
_NCORES = 8
_P = 128
_FREE = _TOTAL // _NCORES // _P     # 32_768 f32 per partition per tensor
_BIG = [4096] * 6 + [2048, 2048]        # cycled through _NBUF big slots
_TAPER = [1024, 1024, 1024, 768, 256]   # dedicated slot each (never recycled)
_TILES = _BIG + _TAPER                  # sums to 32_768
_T = len(_TILES)
_NTAP = len(_TAPER)
_TMAX = 4096
_NBUF = 4
_DIVISOR = float(_TOTAL // 4)       # 8_388_608  (elements per subband)

_CACHE = {}


def _build_nc():
    from contextlib import ExitStack
    import concourse.bass as bass
    import concourse.mybir as mybir

    f32 = mybir.dt.float32
    nc = bass.Bass("TRN2", target_bir_lowering=False)
    x = nc.dram_tensor("x", [_P, _FREE], f32, kind="ExternalInput")
    y = nc.dram_tensor("y", [_P, _FREE], f32, kind="ExternalInput")
    out = nc.dram_tensor("out", [_P, _T], f32, kind="ExternalOutput")

    ctx = ExitStack()
    nc._ctx = ctx  # keep SBUF/semaphore handles alive for compile
    slots = [ctx.enter_context(nc.sbuf_tensor(f"slot{i}", [_P, 2, _TMAX], f32))
             for i in range(_NBUF)]
    taps = [ctx.enter_context(nc.sbuf_tensor(f"tap{i}", [_P, 2, F], f32))
            for i, F in enumerate(_TAPER)]
    stats = ctx.enter_context(nc.sbuf_tensor("stats", [_P, _T], f32))
    dma_sems = [ctx.enter_context(nc.semaphore(f"dma{i}"))
                for i in range(_NBUF)]
    tap_sems = [ctx.enter_context(nc.semaphore(f"tap{i}"))
                for i in range(_NTAP)]
    dve_sem = ctx.enter_context(nc.semaphore("dve"))
    act_sem = ctx.enter_context(nc.semaphore("act"))
    st_sem = ctx.enter_context(nc.semaphore("st"))
    block = ctx.enter_context(nc.Block(no_gpsimd_drain=True))

    offs = np.concatenate(([0], np.cumsum(_TILES))).tolist()
    nbig = len(_BIG)

    def tile_buf_sem(t):
        """(sbuf view, dma sem, dma threshold) for tile t"""
        if t < nbig:
            return slots[t % _NBUF], dma_sems[t % _NBUF], 32 * (t // _NBUF + 1)
        return taps[t - nbig], tap_sems[t - nbig], 32

    @block.sync
    def _(sync):
        # bulk of the stats store overlaps the taper tail
        sync.wait_ge(act_sem, nbig)
        sync.dma_start(out=out[:, :nbig], in_=stats[:, :nbig]) \
            .then_inc(st_sem, 16)
        sync.wait_ge(act_sem, _T)
        sync.dma_start(out=out[:, nbig:], in_=stats[:, nbig:]) \
            .then_inc(st_sem, 16)
        sync.wait_ge(st_sem, 32)  # both stores landed

    @block.vector
    def _(vector):
        for t, F in enumerate(_TILES):
            buf, sem, thr = tile_buf_sem(t)
            vector.wait_ge(sem, thr)
            vector.tensor_sub(
                buf[:, 0, :F], buf[:, 0, :F], buf[:, 1, :F]
            ).then_inc(dve_sem, 1)

    @block.scalar
    def _(scalar):
        def load(t):
            F = _TILES[t]
            buf, sem, _thr = tile_buf_sem(t)
            o = offs[t]
            # ACT-ring HWDGE loads; slot-recycle safety for tiles >= _NBUF
            # is implicit: the dispatch sits after square(t - _NBUF) in this
            # engine's own program order.
            scalar.dma_start(out=buf[:, 0, :F], in_=x[:, o:o + F]) \
                .then_inc(sem, 16)
            scalar.dma_start(out=buf[:, 1, :F], in_=y[:, o:o + F]) \
                .then_inc(sem, 16)

        for t in range(_NBUF):
            load(t)
        for t, F in enumerate(_TILES):
            buf, _sem, _thr = tile_buf_sem(t)
            scalar.wait_ge(dve_sem, t + 1)
            scalar.activation(
                buf[:, 0, :F], buf[:, 0, :F],
                mybir.ActivationFunctionType.Square,
                accum_out=stats[:, t:t + 1],
            ).then_inc(act_sem, 1)
            # square(t) freed slot t % _NBUF -> refill it now; after the
            # last big refill, enqueue the taper so it arrives last.
            if t + _NBUF < nbig:
                load(t + _NBUF)
            elif t + _NBUF == nbig:
                load(t + _NBUF)
                for tt in range(nbig + 1, _T):
                    load(tt)

    ctx.close()
    return nc


def _run(in_maps, trace=False):
    from concourse.bass_utils import run_bass_kernel_spmd

    if "nc" not in _CACHE:
        _CACHE["nc"] = _build_nc()
    return run_bass_kernel_spmd(
        _CACHE["nc"], in_maps, list(range(_NCORES)), trace=trace
    )


def _make_in_maps(input, target):
    xs = np.ascontiguousarray(np.asarray(input, dtype=np.float32)) \
        .reshape(_NCORES, _P, _FREE)
    ys = np.ascontiguousarray(np.asarray(target, dtype=np.float32)) \
        .reshape(_NCORES, _P, _FREE)
    return [{"x": xs[c], "y": ys[c]} for c in range(_NCORES)]


def _finish(results):
    total = 0.0
    for r in results:
        total += r["out"].astype(np.float64).sum()
    return np.array(total / _DIVISOR, dtype=np.float32)


def kernel(input, target):
    res = _run(_make_in_maps(input, target), trace=False)
    return _finish(res.results)
